# revision 13
# baseline (speedup 1.0000x reference)
"""Trainium2 Bass kernel for causal MHA (nn_MHA_18743237280339).

Full-input contract: kernel(**inputs) takes the unsharded numpy inputs and
returns the full [2, 4096, 512] output.

Distribution (8 NeuronCores, SPMD single program):
  - tensor-parallel over (batch, head): core i handles batch b=i//4 and
    heads h0=2*(i%4), h0+1. Projections use host-sliced weight columns, so
    every core runs an identical program on different data.
  - attention is flash-style: scores stay in PSUM/SBUF, softmax denominator
    comes free from a ones-augmented V column (M=65 PV matmul), no
    max-subtraction (logits are tiny for this problem's scale).
  - the scores->exp->PV chain is software-pipelined: the next group's QK^T
    matmuls are issued before the previous group's PV so TensorE streams
    while ScalarE runs exp; the two head-pairs' QK^T matmuls are row-packed
    onto the 128x128 PE array (contraction 64 deep, base partitions 0/64).
  - four intra-program AllGathers (8-core group, bf16, Shared outputs) of
    the per-head attention outputs fire as each quarter completes; each core
    then computes the output projection for a 64-column d_out slice of its
    batch, with wo_b and the folded wv_b bias added there.

Host-side work is limited to slicing/transposing/casting inputs and
reassembling the output.
"""

import math

import numpy as np
import ml_dtypes

import concourse.bass as bass
import concourse.bacc as bacc
import concourse.tile as tile
from concourse import mybir
from concourse.bass_utils import run_bass_kernel_spmd

BF16 = mybir.dt.bfloat16
F32 = mybir.dt.float32

D, H, B, S, HD = 512, 8, 2, 4096, 64
P = 128
NKT = D // P  # 4 contraction tiles of 128
NSB = S // 512  # 8 blocks of 512 rows
NQ = 4  # gather granularity: NSB // NQ q-blocks per AllGather
QW = S // NQ
# Exp offload: blocks j >= 1 compute exp for key-chunks 0..4*min(j,DVE_LEVELS)-1
# on VectorE as (x+1)^2/2 (error x^3/6; logits are ~N(0, 0.073)); the missing
# +1/2 per key is restored by a rank-1 correction matmul into the PV
# accumulator using precomputed 0.5*sum_k Vaug per 512-key level.
DVE_LEVELS = 1
RSQRT2 = 0.7071067811865476

_CACHE: dict = {}


def _build_nc(body_reps=1, do_collective=True):
    nc = bacc.Bacc("TRN2", target_bir_lowering=False, debug=False, num_devices=8)

    xT_d = nc.declare_dram_parameter("xT", [D, S], BF16, isOutput=False)
    wq_d = nc.declare_dram_parameter("wqT", [D, P], BF16, isOutput=False)
    wk_d = nc.declare_dram_parameter("wkT", [D, P], BF16, isOutput=False)
    wv_d = nc.declare_dram_parameter("wvT", [D, P], BF16, isOutput=False)
    wo_d = nc.declare_dram_parameter("woT", [D, HD], BF16, isOutput=False)
    bq_d = nc.declare_dram_parameter("bq", [P, 1], F32, isOutput=False)
    bk_d = nc.declare_dram_parameter("bk", [P, 1], F32, isOutput=False)
    wob_d = nc.declare_dram_parameter("wob", [HD, 1], F32, isOutput=False)
    mask_d = nc.declare_dram_parameter("masks", [4, P, 512], BF16, isOutput=False)
    out_d = nc.declare_dram_parameter("outT", [HD, B * S], F32, isOutput=True)

    with tile.TileContext(nc) as tc:
        for r in range(body_reps):
            _build_body(
                tc, xT_d, wq_d, wk_d, wv_d, wo_d, bq_d, bk_d, wob_d, mask_d, out_d,
                tag=f"r{r}", do_collective=do_collective,
            )

    nc.compile()
    return nc


def _build_body(
    tc, xT_d, wq_d, wk_d, wv_d, wo_d, bq_d, bk_d, wob_d, mask_d, out_d, tag="",
    do_collective=True,
):
    nc = tc.nc
    Exp = mybir.ActivationFunctionType.Exp
    GROUP = 2  # score chunks (of 128 keys) per exp batch; nch always even

    with (
        tc.tile_pool(name=f"const{tag}", bufs=1) as const,
        tc.tile_pool(name=f"kqv{tag}", bufs=1) as kqv,
        tc.tile_pool(name=f"dram{tag}", bufs=1, space="DRAM") as dram,
        tc.tile_pool(name=f"xp{tag}", bufs=3) as xp,
        tc.tile_pool(name=f"sp{tag}", bufs=2, space="PSUM") as spp,  # 2x2 banks
        tc.tile_pool(name=f"pv{tag}", bufs=2, space="PSUM") as pvp,  # 2x1 banks
        tc.tile_pool(name=f"pj{tag}", bufs=2, space="PSUM") as pjp,  # 2x1 banks
        tc.tile_pool(name=f"pt{tag}", bufs=4) as ptp,
        tc.tile_pool(name=f"att{tag}", bufs=3) as att,
        tc.tile_pool(name=f"rc{tag}", bufs=2) as rcp,
        tc.tile_pool(name=f"attg{tag}", bufs=2) as attgp,
        tc.tile_pool(name=f"osb{tag}", bufs=3) as osbp,
    ):
        # ---- constants (weights needed by proj_block(0) first; the rest are
        # loaded after its xt DMA is queued so the pipeline starts sooner) ----
        wk_sb = const.tile([P, NKT, P], BF16, name=f"wk{tag}")
        nc.sync.dma_start(wk_sb[:], wk_d[:, :].rearrange("(c p) m -> p c m", p=P))
        wq_sb = const.tile([P, NKT, P], BF16, name=f"wq{tag}")
        nc.sync.dma_start(wq_sb[:], wq_d[:, :].rearrange("(c p) m -> p c m", p=P))
        wv_sb = const.tile([P, NKT, P], BF16, name=f"wv{tag}")
        nc.sync.dma_start(wv_sb[:], wv_d[:, :].rearrange("(c p) m -> p c m", p=P))
        bq_sb = const.tile([P, 1], F32, name=f"bq{tag}")
        nc.sync.dma_start(bq_sb[:], bq_d[:, :])
        bk_sb = const.tile([P, 1], F32, name=f"bk{tag}")
        nc.sync.dma_start(bk_sb[:], bk_d[:, :])
        wo_sb = const.tile([P, NKT, HD], BF16, name=f"wo{tag}")
        wob_sb = const.tile([HD, 1], F32, name=f"wob{tag}")
        mask_sb = const.tile([P, 4, 512], BF16, name=f"mask{tag}")
        ones_sb = const.tile([P, HD], F32, name=f"ones{tag}")
        halfones = const.tile([P, 1], BF16, name=f"half{tag}")
        ones_row = const.tile([1, 512], BF16, name=f"onesr{tag}")
        corrT = const.tile([1, max(1, DVE_LEVELS), 2 * (HD + 1)], BF16, name=f"corrT{tag}")

        # ---- persistent per-core tensors ----
        KT = kqv.tile([P, S], BF16, name=f"KT{tag}")  # 2 heads stacked (64+64)
        QT = kqv.tile([P, S], BF16, name=f"QT{tag}")
        V0 = kqv.tile([P, S // P, HD + 1], BF16, name=f"V0{tag}")
        V1 = kqv.tile([P, S // P, HD + 1], BF16, name=f"V1{tag}")

        def late_consts():
            nc.sync.dma_start(
                wo_sb[:], wo_d[:, :].rearrange("(c p) m -> p c m", p=P)
            )
            nc.sync.dma_start(wob_sb[:], wob_d[:, :])
            nc.sync.dma_start(
                mask_sb[:], mask_d[:, :, :].rearrange("c p q -> p c q")
            )
            nc.vector.memset(ones_sb[:], 1.0)
            nc.vector.memset(V0[:, :, HD : HD + 1], 1.0)
            nc.vector.memset(V1[:, :, HD : HD + 1], 1.0)
            nc.vector.memset(halfones[:], 0.5)
            nc.vector.memset(ones_row[:], 1.0)

        def compute_corr(level):
            """corrT[0, level-1, 65p:65p+65] = 0.5 * sum of Vaug_p rows over
            keys [512(level-1), 512*level)."""
            cps = pjp.tile([P, 512], F32, tag="pj", name=f"cps{tag}_{level}")
            for p, Vp in enumerate((V0, V1)):
                col = (HD + 1) * p
                for ci in range(4):
                    ch = 4 * (level - 1) + ci
                    nc.tensor.matmul(
                        cps[0:1, col : col + HD + 1],
                        lhsT=halfones[:, :],
                        rhs=Vp[:, ch, :],
                        start=(ci == 0),
                        stop=(ci == 3),
                    )
            nc.vector.tensor_copy(
                corrT[0:1, level - 1, :], cps[0:1, 0 : 2 * (HD + 1)]
            )

        cc_in = [
            dram.tile([2, HD, QW], BF16, name=f"cci{h}{tag}") for h in range(NQ)
        ]
        cc_out = [
            dram.tile([2 * H, HD, QW], BF16, name=f"cco{h}{tag}")
            for h in range(NQ)
        ]

        def proj_block(j):
            sl = slice(512 * j, 512 * (j + 1))
            xt = xp.tile([P, NKT, 512], BF16, tag="xt", name=f"xt{tag}_{j}")
            nc.sync.dma_start(
                xt[:], xT_d[:, sl].rearrange("(c p) s -> p c s", p=P)
            )
            pk = pjp.tile([P, 512], F32, tag="pj", name=f"pk{tag}_{j}")
            for kt in range(NKT):
                nc.tensor.matmul(
                    pk[:, :],
                    lhsT=wk_sb[:, kt, :],
                    rhs=xt[:, kt, :],
                    start=(kt == 0),
                    stop=(kt == NKT - 1),
                )
            nc.vector.tensor_scalar_add(KT[:, sl], pk[:, :], bk_sb[:])
            pq = pjp.tile([P, 512], F32, tag="pj", name=f"pq{tag}_{j}")
            for kt in range(NKT):
                nc.tensor.matmul(
                    pq[:, :],
                    lhsT=wq_sb[:, kt, :],
                    rhs=xt[:, kt, :],
                    start=(kt == 0),
                    stop=(kt == NKT - 1),
                )
            nc.vector.tensor_scalar_add(QT[:, sl], pq[:, :], bq_sb[:])
            pvps = pjp.tile([P, 512], F32, tag="pj", name=f"pvp{tag}_{j}")
            for u in range(4):
                for kt in range(NKT):
                    nc.tensor.matmul(
                        pvps[:, P * u : P * (u + 1)],
                        lhsT=xt[:, kt, P * u : P * (u + 1)],
                        rhs=wv_sb[:, kt, :],
                        start=(kt == 0),
                        stop=(kt == NKT - 1),
                    )
            for u in range(4):
                ch = 4 * j + u
                nc.vector.tensor_copy(
                    V0[:, ch, 0:HD], pvps[:, P * u : P * u + HD]
                )
                nc.vector.tensor_copy(
                    V1[:, ch, 0:HD], pvps[:, P * u + HD : P * (u + 1)]
                )

        def attn_block(j):
            """Scores/exp/PV for q-block j, software-pipelined: the two heads'
            scores for key-chunk kc share one [128, 1024] PSUM tile (head p at
            columns 512p), exp'd in a single ACT call; PV of chunk kc is
            issued after scores of chunk kc+1 so PE streams during exp.
            Returns a flush() that issues the final chunk's mask+PV."""
            qsl = slice(512 * j, 512 * (j + 1))
            nch = 4 * (j + 1)
            pv = [
                pvp.tile([P, 512], F32, tag="pv", name=f"pv{tag}_{p}_{j}")
                for p in range(2)
            ]

            def mask_pv(kc, pt_):
                # For diagonal chunk t = kc - 4j only queries >= 128t see any
                # unmasked key; the mask reduces to the 128x128 triangle at
                # query offset 128t (masks[0][:, 0:128] for every t).
                t = kc - 4 * j
                off = max(0, 128 * t)
                if t >= 0:
                    for p in range(2):
                        c0 = 512 * p + off
                        nc.vector.tensor_mul(
                            pt_[:, c0 : c0 + 128],
                            pt_[:, c0 : c0 + 128],
                            mask_sb[:, 0, 0:128],
                        )
                for p in range(2):
                    Vp = V0 if p == 0 else V1
                    nc.tensor.matmul(
                        pv[p][0 : HD + 1, off:512],
                        lhsT=Vp[:, kc, :],
                        rhs=pt_[:, 512 * p + off : 512 * (p + 1)],
                        start=(kc == 0),
                        stop=(kc == nch - 1),
                    )

            pending = None
            for kc in range(nch):
                t = kc - 4 * j
                off = max(0, 128 * t)
                sp = spp.tile([P, 1024], F32, tag="sp", name=f"sp{tag}_{j}_{kc}")
                for p in range(2):
                    base = HD * p
                    nc.tensor.matmul(
                        sp[:, 512 * p + off : 512 * (p + 1)],
                        lhsT=KT[base : base + HD, P * kc : P * (kc + 1)],
                        rhs=QT[base : base + HD, 512 * j + off : 512 * (j + 1)],
                        start=True,
                        stop=True,
                    )
                pt_ = ptp.tile([P, 1024], BF16, tag="pt", name=f"pt{tag}_{j}_{kc}")
                if kc < 4 * min(j, DVE_LEVELS):
                    # VectorE exp approx: (x+1)^2/2; +1/2 restored by corr MM
                    nc.vector.tensor_scalar(
                        pt_[:, :], sp[:, :], 1.0, RSQRT2,
                        mybir.AluOpType.add, mybir.AluOpType.mult,
                    )
                    nc.vector.tensor_mul(pt_[:, :], pt_[:, :], pt_[:, :])
                elif off == 0:
                    nc.scalar.activation(pt_[:, :], sp[:, :], Exp)
                else:
                    for p in range(2):
                        c0 = 512 * p + off
                        nc.scalar.activation(
                            pt_[:, c0:c0 + 512 - off], sp[:, c0:c0 + 512 - off], Exp
                        )
                if pending is not None:
                    mask_pv(*pending)
                pending = (kc, pt_)

            for level in range(1, min(j, DVE_LEVELS) + 1):
                for p in range(2):
                    col = (HD + 1) * p
                    nc.tensor.matmul(
                        pv[p][0 : HD + 1, :],
                        lhsT=corrT[0:1, level - 1, col : col + HD + 1],
                        rhs=ones_row[0:1, :],
                        start=False,
                        stop=False,
                    )

            def flush():
                mask_pv(*pending)

            return pv, flush

        def norm_store(j, pv):
            for p in range(2):
                rc = rcp.tile([P, 512], F32, tag="rc", name=f"rc{tag}_{p}_{j}")
                nc.vector.reciprocal(rc[HD : HD + 1, :], pv[p][HD : HD + 1, :])
                rb = pjp.tile([P, 512], F32, tag="pj", name=f"rb{tag}_{p}_{j}")
                nc.tensor.matmul(
                    rb[0:HD, :],
                    lhsT=ones_sb[HD : HD + 1, 0:HD],
                    rhs=rc[HD : HD + 1, :],
                    start=True,
                    stop=True,
                )
                rbs = rcp.tile([HD, 512], F32, tag="rbs", name=f"rbs{tag}_{p}_{j}")
                nc.vector.tensor_copy(rbs[:], rb[0:HD, :])
                st = att.tile([HD, 512], BF16, tag="st", name=f"st{tag}_{p}_{j}")
                nc.vector.tensor_mul(st[:], pv[p][0:HD, :], rbs[:])
                quarter, col = divmod(512 * j, QW)
                nc.sync.dma_start(cc_in[quarter][p, :, col : col + 512], st[:])

        def gather_wo(q):
            if do_collective:
                nc.gpsimd.collective_compute(
                    "AllGather",
                    mybir.AluOpType.bypass,
                    replica_groups=[[0, 1, 2, 3, 4, 5, 6, 7]],
                    ins=[cc_in[q][:].opt()],
                    outs=[cc_out[q][:].opt()],
                )
            # cc_out[q] viewed as [b, h, 64, s]: slot 8*b + h
            attg = attgp.tile([P, B * NKT, QW], BF16, tag="attg", name=f"ag{tag}_{q}")
            for b in range(B):
                for c in range(NKT):
                    nc.sync.dma_start(
                        attg[:, NKT * b + c, :],
                        cc_out[q][8 * b + 2 * c : 8 * b + 2 * c + 2, :, :].rearrange(
                            "h p s -> (h p) s"
                        ),
                    )
            for b in range(B):
                for jh in range(QW // 512):
                    ssl = slice(512 * jh, 512 * (jh + 1))
                    osl = slice(
                        S * b + QW * q + 512 * jh,
                        S * b + QW * q + 512 * (jh + 1),
                    )
                    pw = pjp.tile([P, 512], F32, tag="pj", name=f"pw{tag}_{q}_{b}_{jh}")
                    for c in range(NKT):
                        nc.tensor.matmul(
                            pw[0:HD, :],
                            lhsT=wo_sb[:, c, :],
                            rhs=attg[:, NKT * b + c, ssl],
                            start=(c == 0),
                            stop=(c == NKT - 1),
                        )
                    ot = osbp.tile([HD, 512], F32, tag="ot", name=f"ot{tag}_{q}_{b}_{jh}")
                    nc.vector.tensor_scalar_add(ot[:], pw[0:HD, :], wob_sb[:])
                    nc.sync.dma_start(out_d[:, osl], ot[:])

        per_q = NSB // NQ
        proj_block(0)
        late_consts()
        for level in range(1, DVE_LEVELS + 1):
            compute_corr(level)
        for j in range(NSB):
            pv, flush = attn_block(j)
            if j + 1 < NSB:
                proj_block(j + 1)
            flush()
            norm_store(j, pv)
            if (j + 1) % per_q == 0:
                gather_wo((j + 1) // per_q - 1)


def _get_nc():
    if "nc" not in _CACHE:
        _CACHE["nc"] = _build_nc()
    return _CACHE["nc"]


def _prepare_in_maps(x, wq_w, wq_b, wk_w, wk_b, wv_w, wv_b, wo_w, wo_b):
    bf16 = ml_dtypes.bfloat16
    f32 = np.float32
    x = np.asarray(x, f32)
    wq_w = np.asarray(wq_w, f32)
    wq_b = np.asarray(wq_b, f32)
    wk_w = np.asarray(wk_w, f32)
    wk_b = np.asarray(wk_b, f32)
    wv_w = np.asarray(wv_w, f32)
    wv_b = np.asarray(wv_b, f32)
    wo_w = np.asarray(wo_w, f32)
    wo_b = np.asarray(wo_b, f32)

    scale = f32(1.0 / math.sqrt(D))
    wo_b_eff = wo_b + wo_w @ wv_b

    qi = np.arange(512)[None, :]
    ki = np.arange(P)[:, None]
    masks = np.stack(
        [(ki + 128 * c <= qi).astype(f32) for c in range(4)], axis=0
    )  # [4,128,512]
    masks_bf = np.ascontiguousarray(masks.astype(bf16))

    xT = [np.ascontiguousarray(x[b].T).astype(bf16) for b in range(B)]

    in_maps = []
    for i in range(8):
        b = i // 4
        h0 = 2 * (i % 4)
        hs = slice(64 * h0, 64 * h0 + 128)
        cs = slice(64 * i, 64 * (i + 1))
        in_maps.append(
            {
                "xT": xT[b],
                "wqT": np.ascontiguousarray((wq_w[hs, :] * scale).T).astype(bf16),
                "wkT": np.ascontiguousarray(wk_w[hs, :].T).astype(bf16),
                "wvT": np.ascontiguousarray(wv_w[hs, :].T).astype(bf16),
                "woT": np.ascontiguousarray(wo_w[cs, :].T).astype(bf16),
                "bq": np.ascontiguousarray((wq_b[hs] * scale).reshape(P, 1)),
                "bk": np.ascontiguousarray(wk_b[hs].reshape(P, 1)),
                "wob": np.ascontiguousarray(wo_b_eff[cs].reshape(HD, 1)),
                "masks": masks_bf,
            }
        )
    return in_maps


def kernel(
    x, wq_w, wq_b, wk_w, wk_b, wv_w, wv_b, wo_w, wo_b, trace=False, **run_kwargs
):
    in_maps = _prepare_in_maps(x, wq_w, wq_b, wk_w, wk_b, wv_w, wv_b, wo_w, wo_b)
    res = run_bass_kernel_spmd(
        _get_nc(), in_maps, core_ids=list(range(8)), trace=trace, **run_kwargs
    )
    _CACHE["last_result"] = res
    out = np.zeros((B, S, D), np.float32)
    for i in range(8):
        oT = res.results[i]["outT"]  # [64, B*S]
        for b in range(B):
            out[b, :, 64 * i : 64 * (i + 1)] = oT[:, S * b : S * (b + 1)].T
    return out


# revision 16
# speedup vs baseline: 1.3132x; 1.3132x over previous
"""Trainium2 Bass kernel for causal MHA (nn_MHA_18743237280339).

Full-input contract: kernel(**inputs) takes the unsharded numpy inputs and
returns the full [2, 4096, 512] output.

Distribution (8 NeuronCores, SPMD single program):
  - tensor-parallel over (batch, head): core i handles batch b=i//4 and
    heads h0=2*(i%4), h0+1. Projections use host-sliced weight columns, so
    every core runs an identical program on different data.
  - attention is flash-style: scores stay in PSUM/SBUF, softmax denominator
    comes free from a ones-augmented V column (M=65 PV matmul), no
    max-subtraction (logits are tiny for this problem's scale).
  - the scores->exp->PV chain is software-pipelined: the next group's QK^T
    matmuls are issued before the previous group's PV so TensorE streams
    while ScalarE runs exp; the two head-pairs' QK^T matmuls are row-packed
    onto the 128x128 PE array (contraction 64 deep, base partitions 0/64).
  - four intra-program AllGathers (8-core group, bf16, Shared outputs) of
    the per-head attention outputs fire as each quarter completes; each core
    then computes the output projection for a 64-column d_out slice of its
    batch, with wo_b and the folded wv_b bias added there.

Host-side work is limited to slicing/transposing/casting inputs and
reassembling the output.
"""

import math

import numpy as np
import ml_dtypes

import concourse.bass as bass
import concourse.bacc as bacc
import concourse.tile as tile
from concourse import mybir
from concourse.bass_utils import run_bass_kernel_spmd

BF16 = mybir.dt.bfloat16
F32 = mybir.dt.float32

D, H, B, S, HD = 512, 8, 2, 4096, 64
P = 128
NKT = D // P  # 4 contraction tiles of 128
NSB = S // 512  # 8 blocks of 512 rows
NQ = 4  # gather granularity: NSB // NQ q-blocks per AllGather
QW = S // NQ
# Exp offload: blocks j >= 1 compute exp for key-chunks 0..4*min(j,DVE_LEVELS)-1
# on VectorE as (x+1)^2/2 (error x^3/6; logits are ~N(0, 0.073)); the missing
# +1/2 per key is restored by a rank-1 correction matmul into the PV
# accumulator using precomputed 0.5*sum_k Vaug per 512-key level.
DVE_LEVELS = 0
RSQRT2 = 0.7071067811865476

_CACHE: dict = {}


def _build_nc(body_reps=1, do_collective=True, dve_levels=DVE_LEVELS):
    nc = bacc.Bacc("TRN2", target_bir_lowering=False, debug=False, num_devices=8)

    xT_d = nc.declare_dram_parameter("xT", [D, S], BF16, isOutput=False)
    wq_d = nc.declare_dram_parameter("wqT", [D, P], BF16, isOutput=False)
    wk_d = nc.declare_dram_parameter("wkT", [D, P], BF16, isOutput=False)
    wv_d = nc.declare_dram_parameter("wvT", [D, P], BF16, isOutput=False)
    wo_d = nc.declare_dram_parameter("woT", [D, HD], BF16, isOutput=False)
    bq_d = nc.declare_dram_parameter("bq", [P, 1], F32, isOutput=False)
    bk_d = nc.declare_dram_parameter("bk", [P, 1], F32, isOutput=False)
    wob_d = nc.declare_dram_parameter("wob", [HD, 1], F32, isOutput=False)
    mask_d = nc.declare_dram_parameter("masks", [4, P, 512], BF16, isOutput=False)
    out_d = nc.declare_dram_parameter("outT", [HD, B * S], F32, isOutput=True)

    with tile.TileContext(nc) as tc:
        for r in range(body_reps):
            _build_body(
                tc, xT_d, wq_d, wk_d, wv_d, wo_d, bq_d, bk_d, wob_d, mask_d, out_d,
                tag=f"r{r}", do_collective=do_collective, dve_levels=dve_levels,
            )

    nc.compile()
    return nc


def _build_body(
    tc, xT_d, wq_d, wk_d, wv_d, wo_d, bq_d, bk_d, wob_d, mask_d, out_d, tag="",
    do_collective=True, dve_levels=DVE_LEVELS,
):
    nc = tc.nc
    Exp = mybir.ActivationFunctionType.Exp
    GROUP = 2  # score chunks (of 128 keys) per exp batch; nch always even

    with (
        tc.tile_pool(name=f"const{tag}", bufs=1) as const,
        tc.tile_pool(name=f"kqv{tag}", bufs=1) as kqv,
        tc.tile_pool(name=f"dram{tag}", bufs=1, space="DRAM") as dram,
        tc.tile_pool(name=f"xp{tag}", bufs=3) as xp,
        tc.tile_pool(name=f"sp{tag}", bufs=2, space="PSUM") as spp,  # 2x2 banks
        tc.tile_pool(name=f"pv{tag}", bufs=2, space="PSUM") as pvp,  # 2x1 banks
        tc.tile_pool(name=f"pj{tag}", bufs=2, space="PSUM") as pjp,  # 2x1 banks
        tc.tile_pool(name=f"pt{tag}", bufs=4) as ptp,
        tc.tile_pool(name=f"att{tag}", bufs=3) as att,
        tc.tile_pool(name=f"rc{tag}", bufs=2) as rcp,
        tc.tile_pool(name=f"attg{tag}", bufs=2) as attgp,
        tc.tile_pool(name=f"osb{tag}", bufs=3) as osbp,
    ):
        # ---- constants (weights needed by proj_block(0) first; the rest are
        # loaded after its xt DMA is queued so the pipeline starts sooner) ----
        wk_sb = const.tile([P, NKT, P], BF16, name=f"wk{tag}")
        nc.sync.dma_start(wk_sb[:], wk_d[:, :].rearrange("(c p) m -> p c m", p=P))
        wq_sb = const.tile([P, NKT, P], BF16, name=f"wq{tag}")
        nc.sync.dma_start(wq_sb[:], wq_d[:, :].rearrange("(c p) m -> p c m", p=P))
        wv_sb = const.tile([P, NKT, P], BF16, name=f"wv{tag}")
        nc.sync.dma_start(wv_sb[:], wv_d[:, :].rearrange("(c p) m -> p c m", p=P))
        bq_sb = const.tile([P, 1], F32, name=f"bq{tag}")
        nc.sync.dma_start(bq_sb[:], bq_d[:, :])
        bk_sb = const.tile([P, 1], F32, name=f"bk{tag}")
        nc.sync.dma_start(bk_sb[:], bk_d[:, :])
        wo_sb = const.tile([P, NKT, HD], BF16, name=f"wo{tag}")
        wob_sb = const.tile([HD, 1], F32, name=f"wob{tag}")
        mask_sb = const.tile([P, 4, 512], BF16, name=f"mask{tag}")
        ones_sb = const.tile([P, HD], F32, name=f"ones{tag}")
        halfones = const.tile([P, 1], BF16, name=f"half{tag}")
        ones_row = const.tile([1, 512], BF16, name=f"onesr{tag}")
        corrT = const.tile([1, max(1, DVE_LEVELS), 2 * (HD + 1)], BF16, name=f"corrT{tag}")

        # ---- persistent per-core tensors ----
        KT = kqv.tile([P, S], BF16, name=f"KT{tag}")  # 2 heads stacked (64+64)
        QT = kqv.tile([P, S], BF16, name=f"QT{tag}")
        V0 = kqv.tile([P, S // P, HD + 1], BF16, name=f"V0{tag}")
        V1 = kqv.tile([P, S // P, HD + 1], BF16, name=f"V1{tag}")

        def late_consts():
            nc.sync.dma_start(
                wo_sb[:], wo_d[:, :].rearrange("(c p) m -> p c m", p=P)
            )
            nc.sync.dma_start(wob_sb[:], wob_d[:, :])
            nc.sync.dma_start(
                mask_sb[:], mask_d[:, :, :].rearrange("c p q -> p c q")
            )
            nc.vector.memset(ones_sb[:], 1.0)
            nc.vector.memset(V0[:, :, HD : HD + 1], 1.0)
            nc.vector.memset(V1[:, :, HD : HD + 1], 1.0)
            nc.vector.memset(halfones[:], 0.5)
            nc.vector.memset(ones_row[:], 1.0)

        def compute_corr(level):
            """corrT[0, level-1, 65p:65p+65] = 0.5 * sum of Vaug_p rows over
            keys [512(level-1), 512*level)."""
            cps = pjp.tile([P, 512], F32, tag="pj", name=f"cps{tag}_{level}")
            for p, Vp in enumerate((V0, V1)):
                col = (HD + 1) * p
                for ci in range(4):
                    ch = 4 * (level - 1) + ci
                    nc.tensor.matmul(
                        cps[0:1, col : col + HD + 1],
                        lhsT=halfones[:, :],
                        rhs=Vp[:, ch, :],
                        start=(ci == 0),
                        stop=(ci == 3),
                    )
            nc.vector.tensor_copy(
                corrT[0:1, level - 1, :], cps[0:1, 0 : 2 * (HD + 1)]
            )

        cc_in = [
            dram.tile([2, HD, QW], BF16, name=f"cci{h}{tag}") for h in range(NQ)
        ]
        cc_out = [
            dram.tile([2 * H, HD, QW], BF16, name=f"cco{h}{tag}")
            for h in range(NQ)
        ]

        def proj_block(j):
            sl = slice(512 * j, 512 * (j + 1))
            xt = xp.tile([P, NKT, 512], BF16, tag="xt", name=f"xt{tag}_{j}")
            nc.sync.dma_start(
                xt[:], xT_d[:, sl].rearrange("(c p) s -> p c s", p=P)
            )
            pk = pjp.tile([P, 512], F32, tag="pj", name=f"pk{tag}_{j}")
            for kt in range(NKT):
                nc.tensor.matmul(
                    pk[:, :],
                    lhsT=wk_sb[:, kt, :],
                    rhs=xt[:, kt, :],
                    start=(kt == 0),
                    stop=(kt == NKT - 1),
                )
            nc.vector.tensor_scalar_add(KT[:, sl], pk[:, :], bk_sb[:])
            pq = pjp.tile([P, 512], F32, tag="pj", name=f"pq{tag}_{j}")
            for kt in range(NKT):
                nc.tensor.matmul(
                    pq[:, :],
                    lhsT=wq_sb[:, kt, :],
                    rhs=xt[:, kt, :],
                    start=(kt == 0),
                    stop=(kt == NKT - 1),
                )
            nc.vector.tensor_scalar_add(QT[:, sl], pq[:, :], bq_sb[:])
            pvps = pjp.tile([P, 512], F32, tag="pj", name=f"pvp{tag}_{j}")
            for u in range(4):
                for kt in range(NKT):
                    nc.tensor.matmul(
                        pvps[:, P * u : P * (u + 1)],
                        lhsT=xt[:, kt, P * u : P * (u + 1)],
                        rhs=wv_sb[:, kt, :],
                        start=(kt == 0),
                        stop=(kt == NKT - 1),
                    )
            for u in range(4):
                ch = 4 * j + u
                nc.vector.tensor_copy(
                    V0[:, ch, 0:HD], pvps[:, P * u : P * u + HD]
                )
                nc.vector.tensor_copy(
                    V1[:, ch, 0:HD], pvps[:, P * u + HD : P * (u + 1)]
                )

        def attn_block(j):
            """Scores/exp/PV for q-block j, software-pipelined: the two heads'
            scores for key-chunk kc share one [128, 1024] PSUM tile (head p at
            columns 512p), exp'd in a single ACT call; PV of chunk kc is
            issued after scores of chunk kc+1 so PE streams during exp.
            Returns a flush() that issues the final chunk's mask+PV."""
            qsl = slice(512 * j, 512 * (j + 1))
            nch = 4 * (j + 1)
            pv = [
                pvp.tile([P, 512], F32, tag="pv", name=f"pv{tag}_{p}_{j}")
                for p in range(2)
            ]

            def mask_pv(kc, pt_):
                # For diagonal chunk t = kc - 4j only queries >= 128t see any
                # unmasked key; the mask reduces to the 128x128 triangle at
                # query offset 128t (masks[0][:, 0:128] for every t).
                t = kc - 4 * j
                off = max(0, 128 * t)
                if t >= 0:
                    for p in range(2):
                        c0 = 512 * p + off
                        nc.vector.tensor_mul(
                            pt_[:, c0 : c0 + 128],
                            pt_[:, c0 : c0 + 128],
                            mask_sb[:, 0, 0:128],
                        )
                for p in range(2):
                    Vp = V0 if p == 0 else V1
                    nc.tensor.matmul(
                        pv[p][0 : HD + 1, off:512],
                        lhsT=Vp[:, kc, :],
                        rhs=pt_[:, 512 * p + off : 512 * (p + 1)],
                        start=(kc == 0),
                        stop=(kc == nch - 1),
                    )

            pending = None
            for kc in range(nch):
                t = kc - 4 * j
                off = max(0, 128 * t)
                sp = spp.tile([P, 1024], F32, tag="sp", name=f"sp{tag}_{j}_{kc}")
                for p in range(2):
                    base = HD * p
                    nc.tensor.matmul(
                        sp[:, 512 * p + off : 512 * (p + 1)],
                        lhsT=KT[base : base + HD, P * kc : P * (kc + 1)],
                        rhs=QT[base : base + HD, 512 * j + off : 512 * (j + 1)],
                        start=True,
                        stop=True,
                    )
                pt_ = ptp.tile([P, 1024], BF16, tag="pt", name=f"pt{tag}_{j}_{kc}")
                if kc < 4 * min(j, dve_levels):
                    # VectorE exp approx: (x+1)^2/2; +1/2 restored by corr MM
                    nc.vector.tensor_scalar(
                        pt_[:, :], sp[:, :], 1.0, RSQRT2,
                        mybir.AluOpType.add, mybir.AluOpType.mult,
                    )
                    nc.vector.tensor_mul(pt_[:, :], pt_[:, :], pt_[:, :])
                elif off == 0:
                    nc.scalar.activation(pt_[:, :], sp[:, :], Exp)
                else:
                    for p in range(2):
                        c0 = 512 * p + off
                        nc.scalar.activation(
                            pt_[:, c0:c0 + 512 - off], sp[:, c0:c0 + 512 - off], Exp
                        )
                if pending is not None:
                    mask_pv(*pending)
                pending = (kc, pt_)

            for level in range(1, min(j, dve_levels) + 1):
                for p in range(2):
                    col = (HD + 1) * p
                    nc.tensor.matmul(
                        pv[p][0 : HD + 1, :],
                        lhsT=corrT[0:1, level - 1, col : col + HD + 1],
                        rhs=ones_row[0:1, :],
                        start=False,
                        stop=False,
                    )

            def flush():
                mask_pv(*pending)

            return pv, flush

        def norm_store(j, pv):
            for p in range(2):
                rc = rcp.tile([P, 512], F32, tag="rc", name=f"rc{tag}_{p}_{j}")
                nc.vector.reciprocal(rc[HD : HD + 1, :], pv[p][HD : HD + 1, :])
                rb = pjp.tile([P, 512], F32, tag="pj", name=f"rb{tag}_{p}_{j}")
                nc.tensor.matmul(
                    rb[0:HD, :],
                    lhsT=ones_sb[HD : HD + 1, 0:HD],
                    rhs=rc[HD : HD + 1, :],
                    start=True,
                    stop=True,
                )
                rbs = rcp.tile([HD, 512], F32, tag="rbs", name=f"rbs{tag}_{p}_{j}")
                nc.vector.tensor_copy(rbs[:], rb[0:HD, :])
                st = att.tile([HD, 512], BF16, tag="st", name=f"st{tag}_{p}_{j}")
                nc.vector.tensor_mul(st[:], pv[p][0:HD, :], rbs[:])
                quarter, col = divmod(512 * j, QW)
                nc.sync.dma_start(cc_in[quarter][p, :, col : col + 512], st[:])

        def gather_wo(q):
            if do_collective:
                nc.gpsimd.collective_compute(
                    "AllGather",
                    mybir.AluOpType.bypass,
                    replica_groups=[[0, 1, 2, 3, 4, 5, 6, 7]],
                    ins=[cc_in[q][:].opt()],
                    outs=[cc_out[q][:].opt()],
                )
            # cc_out[q] viewed as [b, h, 64, s]: slot 8*b + h
            attg = attgp.tile([P, B * NKT, QW], BF16, tag="attg", name=f"ag{tag}_{q}")
            for b in range(B):
                for c in range(NKT):
                    nc.sync.dma_start(
                        attg[:, NKT * b + c, :],
                        cc_out[q][8 * b + 2 * c : 8 * b + 2 * c + 2, :, :].rearrange(
                            "h p s -> (h p) s"
                        ),
                    )
            for b in range(B):
                for jh in range(QW // 512):
                    ssl = slice(512 * jh, 512 * (jh + 1))
                    osl = slice(
                        S * b + QW * q + 512 * jh,
                        S * b + QW * q + 512 * (jh + 1),
                    )
                    pw = pjp.tile([P, 512], F32, tag="pj", name=f"pw{tag}_{q}_{b}_{jh}")
                    for c in range(NKT):
                        nc.tensor.matmul(
                            pw[0:HD, :],
                            lhsT=wo_sb[:, c, :],
                            rhs=attg[:, NKT * b + c, ssl],
                            start=(c == 0),
                            stop=(c == NKT - 1),
                        )
                    ot = osbp.tile([HD, 512], F32, tag="ot", name=f"ot{tag}_{q}_{b}_{jh}")
                    nc.vector.tensor_scalar_add(ot[:], pw[0:HD, :], wob_sb[:])
                    nc.sync.dma_start(out_d[:, osl], ot[:])

        per_q = NSB // NQ
        proj_block(0)
        late_consts()
        for level in range(1, dve_levels + 1):
            compute_corr(level)
        for j in range(NSB):
            pv, flush = attn_block(j)
            if j + 1 < NSB:
                proj_block(j + 1)
            flush()
            norm_store(j, pv)
            if (j + 1) % per_q == 0:
                gather_wo((j + 1) // per_q - 1)


def _get_nc():
    if "nc" not in _CACHE:
        _CACHE["nc"] = _build_nc()
    return _CACHE["nc"]


def _prepare_in_maps(x, wq_w, wq_b, wk_w, wk_b, wv_w, wv_b, wo_w, wo_b):
    bf16 = ml_dtypes.bfloat16
    f32 = np.float32
    x = np.asarray(x, f32)
    wq_w = np.asarray(wq_w, f32)
    wq_b = np.asarray(wq_b, f32)
    wk_w = np.asarray(wk_w, f32)
    wk_b = np.asarray(wk_b, f32)
    wv_w = np.asarray(wv_w, f32)
    wv_b = np.asarray(wv_b, f32)
    wo_w = np.asarray(wo_w, f32)
    wo_b = np.asarray(wo_b, f32)

    scale = f32(1.0 / math.sqrt(D))
    wo_b_eff = wo_b + wo_w @ wv_b

    qi = np.arange(512)[None, :]
    ki = np.arange(P)[:, None]
    masks = np.stack(
        [(ki + 128 * c <= qi).astype(f32) for c in range(4)], axis=0
    )  # [4,128,512]
    masks_bf = np.ascontiguousarray(masks.astype(bf16))

    xT = [np.ascontiguousarray(x[b].T).astype(bf16) for b in range(B)]

    in_maps = []
    for i in range(8):
        b = i // 4
        h0 = 2 * (i % 4)
        hs = slice(64 * h0, 64 * h0 + 128)
        cs = slice(64 * i, 64 * (i + 1))
        in_maps.append(
            {
                "xT": xT[b],
                "wqT": np.ascontiguousarray((wq_w[hs, :] * scale).T).astype(bf16),
                "wkT": np.ascontiguousarray(wk_w[hs, :].T).astype(bf16),
                "wvT": np.ascontiguousarray(wv_w[hs, :].T).astype(bf16),
                "woT": np.ascontiguousarray(wo_w[cs, :].T).astype(bf16),
                "bq": np.ascontiguousarray((wq_b[hs] * scale).reshape(P, 1)),
                "bk": np.ascontiguousarray(wk_b[hs].reshape(P, 1)),
                "wob": np.ascontiguousarray(wo_b_eff[cs].reshape(HD, 1)),
                "masks": masks_bf,
            }
        )
    return in_maps


def kernel(
    x, wq_w, wq_b, wk_w, wk_b, wv_w, wv_b, wo_w, wo_b, trace=False, **run_kwargs
):
    in_maps = _prepare_in_maps(x, wq_w, wq_b, wk_w, wk_b, wv_w, wv_b, wo_w, wo_b)
    res = run_bass_kernel_spmd(
        _get_nc(), in_maps, core_ids=list(range(8)), trace=trace, **run_kwargs
    )
    _CACHE["last_result"] = res
    out = np.zeros((B, S, D), np.float32)
    for i in range(8):
        oT = res.results[i]["outT"]  # [64, B*S]
        for b in range(B):
            out[b, :, 64 * i : 64 * (i + 1)] = oT[:, S * b : S * (b + 1)].T
    return out
